# revision 19
# baseline (speedup 1.0000x reference)
"""CoGNN forward pass on 8 Trainium2 NeuronCores.

Strategy: shard nodes contiguously across cores (6250/core); assign each edge
to the core owning its destination node. Per core, destination nodes are
processed in 49 tiles of 128; each tile's incoming edges are split into
A (u < 32768) and B (u >= 32768) runs (dma_gather indices are int16), padded
to fixed chunk counts (CA/CB chunks of 128 edges) uniform across cores so one
SPMD program serves all 8 cores.

Per conv: gather source-node rows with dma_gather from a DRAM table
(AllGather-replicated each layer), build one-hot [128edges x 128nodes]
matrices from precomputed local-v values, and segment-sum via PE matmuls
accumulated in PSUM. Static edge-attr contributions (segment_sum of
relu(edge_attr@W)) are computed once in a prologue. Edge gating (gumbel-hard)
reduces to hard thresholds with host-precomputed gumbel noise (fixed seed).
The weighted conv gathers 512B rows [hn | ok] so the source keep-gate rides
along with the features; the dest keep-gate scales whole rows after the
segment sum.
"""
import numpy as np

N = 50000  # noqa: E305
E = 800000
G = 64
ENV = 64
ACT = 16
D = 64
F_NODE = 153
NC = 8
NPC = N // NC          # 6250 nodes per core
P = 128
NT = 49                # node tiles per core (48*128 + 106)
LAST_ROWS = NPC - 48 * P   # 106
SPLIT = 32768          # A/B split for int16 gather indices
SUB = 4                # tiles per gather sub-batch (256B-row passes)
SUB3 = 2               # tiles per gather sub-batch (512B-row pass)

_cache = {}


def _gumbel_diffs():
    import jax
    import jax.numpy as jnp
    with jax.default_device(jax.devices("cpu")[0]):
        gkey = jax.random.key(42)
        out = []
        for j in range(6):
            g = jax.random.gumbel(jax.random.fold_in(gkey, j), (N, 2), jnp.float32)
            out.append(np.asarray(g[:, 0] - g[:, 1], np.float32))
    return out  # j=2i -> in-gate of layer i, j=2i+1 -> out-gate


def _wrap_idx(vals):
    """int16 values for one dma_gather call -> wrapped [128, len/16] layout."""
    n = len(vals)
    assert n % 16 == 0
    w = np.asarray(vals, np.int16).reshape(n // 16, 16).T  # i -> [i%16, i//16]
    return np.tile(w, (8, 1))


def _sub_batches(n_tiles, sub):
    out = []
    t0 = 0
    while t0 < n_tiles:
        out.append((t0, min(sub, n_tiles - t0)))
        t0 += min(sub, n_tiles - t0)
    return out


def _prep(edge_index, batch):
    """All data-dependent-but-weight-independent host prep, cached."""
    key = edge_index.tobytes()[:64]
    if key in _cache:
        return _cache[key]
    u = edge_index[0].astype(np.int64)
    v = edge_index[1].astype(np.int64)
    owner = v // NPC
    per_core = []
    CA = CB = 0
    for c in range(NC):
        m = owner == c
        uc, vc = u[m], v[m]
        vloc = vc - c * NPC
        t = vloc >> 7
        lv = vloc & 127
        isA = uc < SPLIT
        tiles = []
        for ti in range(NT):
            tm = t == ti
            A = np.nonzero(tm & isA)[0]
            B = np.nonzero(tm & ~isA)[0]
            tiles.append((uc[A], lv[A], np.nonzero(m)[0][A],
                          uc[B], lv[B], np.nonzero(m)[0][B]))
            CA = max(CA, (len(A) + 127) // 128)
            CB = max(CB, (len(B) + 127) // 128)
        per_core.append(tiles)
    CPT = CA + CB
    NCH = NT * CPT

    data = {"CA": CA, "CB": CB, "CPT": CPT, "NCH": NCH, "cores": []}
    for c in range(NC):
        tiles = per_core[c]
        lv_all = np.full((P, NCH), -1.0, np.float32)
        uA = np.zeros(NT * CA * 128, np.int64)     # table row index (u), 0-pad
        uB = np.zeros(NT * CB * 128, np.int64)     # u - SPLIT, 0-pad
        eid = np.full(NCH * 128, -1, np.int64)     # original edge id, -1 pad
        for ti in range(NT):
            au, alv, aid, bu, blv, bid = tiles[ti]
            na, nb = len(au), len(bu)
            uA[ti * CA * 128: ti * CA * 128 + na] = au
            uB[ti * CB * 128: ti * CB * 128 + nb] = bu - SPLIT
            for j in range(CA):
                sl = slice(ti * CA * 128 + j * 128, ti * CA * 128 + (j + 1) * 128)
                k = ti * CPT + j
                seg = np.arange(j * 128, min((j + 1) * 128, na))
                lv_all[: len(seg), k] = alv[seg]
                eid[k * 128: k * 128 + len(seg)] = aid[seg]
            for j in range(CB):
                k = ti * CPT + CA + j
                seg = np.arange(j * 128, min((j + 1) * 128, nb))
                lv_all[: len(seg), k] = blv[seg]
                eid[k * 128: k * 128 + len(seg)] = bid[seg]

        # wrapped idx arrays, per sub-batch call, concatenated along cols
        def wrap_stream(uarr, cpt, sub):
            cols = []
            for t0, ns in _sub_batches(NT, sub):
                seg = uarr[t0 * cpt * 128: (t0 + ns) * cpt * 128]
                cols.append(_wrap_idx(seg))
            return np.concatenate(cols, axis=1)

        idxA = wrap_stream(uA, CA, SUB)
        idxB = wrap_stream(uB, CB, SUB)
        idxA3 = wrap_stream(uA, CA, SUB3)
        idxB3 = wrap_stream(uB, CB, SUB3)

        nodes = np.arange(c * NPC, (c + 1) * NPC)
        bt = np.full((P, NT), 999.0, np.float32)
        bt.T.flat[:NPC] = batch[nodes]
        data["cores"].append({
            "lv": lv_all, "idxA": idxA, "idxB": idxB,
            "idxA3": idxA3, "idxB3": idxB3, "eid": eid, "bt": bt,
        })
    _cache[key] = data
    return data


def _build_program(CA, CB, shapes):
    import sys, os
    DBG_LAYERS = int(os.environ.get("CK_LAYERS", "3"))
    DBG_PASSES = os.environ.get("CK_PASSES", "123")  # which passes per layer
    DBG_PRO = os.environ.get("CK_PRO", "1") == "1"
    if "/opt/trn_rl_repo" not in sys.path:
        sys.path.insert(0, "/opt/trn_rl_repo")
    import concourse.bass as bass
    import concourse.mybir as mybir
    import concourse.tile as tile
    from concourse import library_config
    from concourse.library_overlay import lower_extended_insts
    from concourse.masks import make_identity

    def fix_waits(ncx):
        """walrus here encodes at most one sem-wait per instruction and none on
        custom/ISA/dynamic-queue instructions; move extras onto EVSEMs."""
        NO_WAIT = ("InstDMAGatherAnt", "InstDMAScatterAddAnt", "InstCollectiveCompute")
        for fn in ncx.m.functions:
            for blk in fn.blocks:
                new = []
                changed = False
                for ins in list(blk.instructions):
                    tn = type(ins).__name__
                    si = ins.sync_info
                    waits = list(si.on_wait) if si is not None and si.on_wait else []
                    no_wait = (tn in NO_WAIT or isinstance(ins, mybir.InstISA)
                               or (tn == "InstDMACopy"
                                   and getattr(ins, "queue", "") == "qPoolDynamic"))
                    keep = 0 if no_wait else 1
                    if len(waits) <= keep:
                        new.append(ins)
                        continue
                    cut = len(waits) - keep
                    for w in waits[:cut]:
                        new.append(mybir.InstEventSemaphore(
                            name=ncx.get_next_instruction_name(),
                            engine=ins.engine, ins=[], outs=[], bass_nofuse=True,
                            sync_info=mybir.SyncInfo(on_wait=[w], on_update=[])))
                    ins.sync_info = mybir.SyncInfo(
                        on_wait=waits[cut:], on_update=list(si.on_update) if si else [])
                    new.append(ins)
                    changed = True
                if changed:
                    blk.instructions = new

    f32 = mybir.dt.float32
    i16 = mybir.dt.int16
    CPT = CA + CB
    NCH = NT * CPT
    AF = mybir.ActivationFunctionType
    OP = mybir.AluOpType

    nc = bass.Bass(num_devices=NC)

    def din(name, shape, dt=f32):
        return nc.dram_tensor(name, list(shape), dt, kind="ExternalInput")

    t_lv = din("lv", (P, NCH))
    t_idxA = din("idxA", shapes["idxA"], i16)
    t_idxB = din("idxB", shapes["idxB"], i16)
    t_idxA3 = din("idxA3", shapes["idxA3"], i16)
    t_idxB3 = din("idxB3", shapes["idxB3"], i16)
    t_ea = din("ea8", (NCH * 128, 8))
    t_bt = din("bt", (P, NT))
    t_dg = din("dg6", (P, 6 * NT))
    t_xT1 = din("xT1", (P, NT * P))
    t_xT2 = din("xT2", (25, NT * P))
    t_iota = din("iota", (P, P))
    t_Wn1 = din("Wn1", (P, 64))
    t_Wn2 = din("Wn2", (25, 64))
    t_bn = din("bn_t", (P, 64))
    t_W8 = din("W8", (8, 80))
    t_Wa = din("Wa1o1", (P, 32))
    t_ba = din("ba1o1_t", (P, 32))
    t_W5 = din("W5", (65, 4))
    t_Wc = [din(f"Wc{i}", (P, 64)) for i in range(3)]
    t_bc = [din(f"bc{i}_t", (P, 64)) for i in range(3)]
    t_Wd1 = din("Wd1b", (65, 64))
    t_Wd2 = din("Wd2b", (65, 64))
    t_Wg1 = din("Wg1b", (65, 64))
    t_Wg2 = din("Wg2b", (65, 64))
    t_lng = din("lng_t", (P, 64))
    t_lnb = din("lnb_t", (P, 64))

    DBG = os.environ.get("CK_DBG", "")
    o_emb = nc.dram_tensor("o_emb", [NPC, 64], f32, kind="ExternalOutput")
    o_dbg = nc.dram_tensor("o_dbg", [P, NT * 80], f32, kind="ExternalOutput")
    o_pool = nc.dram_tensor("o_pool", [64, 128], f32, kind="ExternalOutput")

    ag1_in = nc.dram_tensor("ag1_in", [NPC, 64], f32, kind="Internal")
    hn_tab = nc.dram_tensor("hn_tab", [N, 64], f32, kind="Internal",
                            addr_space="Shared")
    ag2_in = nc.dram_tensor("ag2_in", [NPC, 64], f32, kind="Internal")
    aio_tab = nc.dram_tensor("aio_tab", [N, 64], f32, kind="Internal",
                             addr_space="Shared")
    ag3_in = nc.dram_tensor("ag3_in", [NPC, 128], f32, kind="Internal")
    w_tab = nc.dram_tensor("w_tab", [N, 128], f32, kind="Internal",
                           addr_space="Shared")
    groups = [list(range(NC))]

    with tile.TileContext(nc) as tc:
        with (
            tc.tile_pool(name="cst", bufs=1) as cst,
            tc.tile_pool(name="st", bufs=1) as st,
            tc.tile_pool(name="gat", bufs=2) as gat,
            tc.tile_pool(name="wk", bufs=3) as wk,
            tc.tile_pool(name="ps_seg", bufs=2, space="PSUM") as ps_seg,
            tc.tile_pool(name="ps_tr", bufs=2, space="PSUM") as ps_tr,
            tc.tile_pool(name="ps_nmm", bufs=2, space="PSUM") as ps_nmm,
            tc.tile_pool(name="ps_pool", bufs=1, space="PSUM") as ps_pool,
        ):
            nc.gpsimd.load_library(library_config.mlp)
            ident = cst.tile([P, P], f32)
            make_identity(nc, ident[:])
            iota = cst.tile([P, P], f32); nc.sync.dma_start(iota[:], t_iota[:])
            lv = cst.tile([P, NCH], f32); nc.sync.dma_start(lv[:], t_lv[:])
            idxA = cst.tile(list(shapes["idxA"]), i16); nc.sync.dma_start(idxA[:], t_idxA[:])
            idxB = cst.tile(list(shapes["idxB"]), i16); nc.sync.dma_start(idxB[:], t_idxB[:])
            idxA3 = cst.tile(list(shapes["idxA3"]), i16); nc.sync.dma_start(idxA3[:], t_idxA3[:])
            idxB3 = cst.tile(list(shapes["idxB3"]), i16); nc.sync.dma_start(idxB3[:], t_idxB3[:])
            bt = cst.tile([P, NT], f32); nc.sync.dma_start(bt[:], t_bt[:])
            dg6 = cst.tile([P, 6 * NT], f32); nc.sync.dma_start(dg6[:], t_dg[:])
            Wn1 = cst.tile([P, 64], f32); nc.sync.dma_start(Wn1[:], t_Wn1[:])
            Wn2 = cst.tile([25, 64], f32); nc.sync.dma_start(Wn2[:], t_Wn2[:])
            bn = cst.tile([P, 64], f32); nc.sync.dma_start(bn[:], t_bn[:])
            W8 = cst.tile([8, 80], f32); nc.sync.dma_start(W8[:], t_W8[:])
            Wa = cst.tile([P, 32], f32); nc.sync.dma_start(Wa[:], t_Wa[:])
            ba = cst.tile([P, 32], f32); nc.sync.dma_start(ba[:], t_ba[:])
            W5 = cst.tile([65, 4], f32); nc.sync.dma_start(W5[:], t_W5[:])
            Wc = []
            bc = []
            for i in range(3):
                w_ = cst.tile([P, 64], f32, tag=f"Wc{i}"); nc.sync.dma_start(w_[:], t_Wc[i][:])
                b_ = cst.tile([P, 64], f32, tag=f"bc{i}"); nc.sync.dma_start(b_[:], t_bc[i][:])
                Wc.append(w_); bc.append(b_)
            Wd1 = cst.tile([65, 64], f32); nc.sync.dma_start(Wd1[:], t_Wd1[:])
            Wd2 = cst.tile([65, 64], f32); nc.sync.dma_start(Wd2[:], t_Wd2[:])
            Wg1 = cst.tile([65, 64], f32); nc.sync.dma_start(Wg1[:], t_Wg1[:])
            Wg2 = cst.tile([65, 64], f32); nc.sync.dma_start(Wg2[:], t_Wg2[:])
            lng = cst.tile([P, 64], f32); nc.sync.dma_start(lng[:], t_lng[:])
            lnb = cst.tile([P, 64], f32); nc.sync.dma_start(lnb[:], t_lnb[:])
            eps = cst.tile([P, 1], f32); nc.any.memset(eps[:], 1e-5)
            neghalf = cst.tile([P, 1], f32); nc.any.memset(neghalf[:], -0.5)
            onep5 = cst.tile([P, 1], f32); nc.any.memset(onep5[:], 1.5)

            nreg = {}
            for _, ns_ in _sub_batches(NT, SUB):
                for v_ in (ns_ * CA * 128, ns_ * CB * 128):
                    nreg.setdefault(v_, None)
            for _, ns_ in _sub_batches(NT, SUB3):
                for v_ in (ns_ * CA * 128, ns_ * CB * 128):
                    nreg.setdefault(v_, None)
            for v_ in nreg:
                nreg[v_] = nc.gpsimd.to_reg(v_)

            h_sb = st.tile([P, NT * 64], f32)
            hn_sb = st.tile([P, NT * 64], f32)
            aio_sb = st.tile([P, NT * 32], f32)
            seg_sb = st.tile([P, NT * 80], f32)
            l4_sb = st.tile([P, NT * 4], f32)
            ik_sb = st.tile([P, NT], f32)
            ok_sb = st.tile([P, NT], f32)
            wt_sb = st.tile([P, NT * 128], f32)

            ea_dram = t_ea[:].rearrange("(k p) f -> p k f", p=P)

            def ea_chunk(eab, kk, k):
                """edge-attr chunk kk of batch eab -> EA=relu([ea|1]@W8) [P,Fw] in SBUF."""
                eT_ps = ps_tr.tile([8, P], f32, tag="tr")
                nc.tensor.transpose(out=eT_ps[:], in_=eab[:, kk, :], identity=ident[:])
                eT = wk.tile([8, P], f32, tag="eT")
                nc.vector.tensor_copy(eT[:], eT_ps[:])
                ea_ps = ps_nmm.tile([P, 80], f32, tag="nmm")
                nc.tensor.matmul(ea_ps[:, :80], eT[:], W8[:], start=True, stop=True)
                EA = wk.tile([P, 80], f32, tag="EA")
                nc.scalar.activation(EA[:], ea_ps[:, :80], AF.Relu)
                return EA

            def onehot(k):
                oh = wk.tile([P, P], f32, tag="oh")
                nc.vector.tensor_tensor(
                    out=oh[:], in0=lv[:, k:k + 1].to_broadcast([P, P]),
                    in1=iota[:], op=OP.is_equal)
                return oh

            def ln_tile(t, dst):
                """dst[:, t*64:+64] = LN(h_sb[:, t*64:+64]) * g + b"""
                hsl = h_sb[:, t * 64:(t + 1) * 64]
                msum = wk.tile([P, 1], f32, tag="ln1")
                nc.vector.reduce_sum(msum[:], hsl, axis=mybir.AxisListType.X)
                meanv = wk.tile([P, 1], f32, tag="ln2")
                nc.scalar.activation(meanv[:], msum[:], AF.Copy, scale=1.0 / 64)
                cent = wk.tile([P, 64], f32, tag="ln3")
                nc.vector.tensor_scalar_sub(cent[:], hsl, meanv[:])
                sq = wk.tile([P, 64], f32, tag="ln4")
                nc.vector.tensor_tensor(out=sq[:], in0=cent[:], in1=cent[:], op=OP.mult)
                vsum = wk.tile([P, 1], f32, tag="ln5")
                nc.vector.reduce_sum(vsum[:], sq[:], axis=mybir.AxisListType.X)
                xv = wk.tile([P, 1], f32, tag="ln6a")
                nc.scalar.activation(xv[:], vsum[:], AF.Copy, scale=1.0 / 64)
                nc.vector.tensor_scalar_add(xv[:], xv[:], eps[:])
                std = wk.tile([P, 1], f32, tag="ln6")
                nc.scalar.activation(std[:], vsum[:], AF.Sqrt, scale=1.0 / 64, bias=eps[:])
                r0 = wk.tile([P, 1], f32, tag="ln7")
                nc.vector.reciprocal(r0[:], std[:])
                # Newton for y ~= 1/sqrt(x): y1 = y0*(1.5 - 0.5*x*y0^2)
                tmp = wk.tile([P, 1], f32, tag="ln8")
                nc.vector.tensor_tensor(out=tmp[:], in0=xv[:], in1=r0[:], op=OP.mult)
                nc.vector.tensor_tensor(out=tmp[:], in0=tmp[:], in1=r0[:], op=OP.mult)
                nc.vector.tensor_scalar(out=tmp[:], in0=tmp[:], scalar1=neghalf[:],
                                        scalar2=onep5[:], op0=OP.mult, op1=OP.add)
                r1 = wk.tile([P, 1], f32, tag="ln9")
                nc.vector.tensor_tensor(out=r1[:], in0=r0[:], in1=tmp[:], op=OP.mult)
                dsl = dst[:, t * 64:(t + 1) * 64]
                nc.vector.tensor_scalar_mul(dsl, cent[:], r1[:])
                nc.vector.tensor_tensor(out=dsl, in0=dsl, in1=lng[:], op=OP.mult)
                nc.vector.tensor_tensor(out=dsl, in0=dsl, in1=lnb[:], op=OP.add)

            def cat_mm(lhs_a, lhs_b, rhs, Fw):
                """psum[P,Fw] = [lhs_a | lhs_b] @ rhs via two transposes."""
                trA = ps_tr.tile([64, P], f32, tag="tr")
                nc.tensor.transpose(out=trA[:], in_=lhs_a, identity=ident[:])
                trB = ps_tr.tile([64, P], f32, tag="tr")
                nc.tensor.transpose(out=trB[:], in_=lhs_b, identity=ident[:])
                catT = wk.tile([P, P], f32, tag="catT")
                nc.vector.tensor_copy(catT[0:64, :], trA[:])
                nc.vector.tensor_copy(catT[64:128, :], trB[:])
                out_ps = ps_nmm.tile([P, 80], f32, tag="nmm")
                nc.tensor.matmul(out_ps[:, :Fw], catT[:], rhs, start=True, stop=True)
                return out_ps

            def mm65(lhs, rhs, Fw):
                """psum[P,Fw] = [lhs | 1] @ rhs; lhs is [P,64] SBUF."""
                trA = ps_tr.tile([64, P], f32, tag="tr")
                nc.tensor.transpose(out=trA[:], in_=lhs, identity=ident[:])
                catT = wk.tile([65, P], f32, tag="catT65")
                nc.vector.tensor_copy(catT[0:64, :], trA[:])
                nc.any.memset(catT[64:65, :], 1.0)
                out_ps = ps_nmm.tile([P, 80], f32, tag="nmm")
                nc.tensor.matmul(out_ps[:, :Fw], catT[:], rhs, start=True, stop=True)
                return out_ps

            def contrib_dma(dram, sb, width, colw):
                """write [NPC, width] rows (cols 0:width) from sb [P, NT*colw]."""
                d3 = dram[0:48 * P, :].rearrange("(t p) f -> p t f", p=P)
                s3 = sb[:].rearrange("p (t f) -> p t f", f=colw)
                nc.sync.dma_start(d3[:, :, 0:colw], s3[:, 0:48, :])
                nc.sync.dma_start(
                    dram[48 * P: 48 * P + LAST_ROWS, 0:colw],
                    sb[0:LAST_ROWS, 48 * colw:(48 + 1) * colw])

            # ---------------- h0 = relu(x @ Wn + bn) ----------------
            for t in range(NT):
                x1 = wk.tile([P, P], f32, tag="x1")
                nc.sync.dma_start(x1[:], t_xT1[:, t * P:(t + 1) * P])
                x2 = wk.tile([25, P], f32, tag="x2")
                nc.sync.dma_start(x2[:], t_xT2[:, t * P:(t + 1) * P])
                h_ps = ps_nmm.tile([P, 80], f32, tag="nmm")
                nc.tensor.matmul(h_ps[:, :64], x1[:], Wn1[:], start=True, stop=False)
                nc.tensor.matmul(h_ps[:, :64], x2[:], Wn2[:], start=False, stop=True)
                hsl = h_sb[:, t * 64:(t + 1) * 64]
                nc.vector.tensor_tensor(out=hsl, in0=h_ps[:, :64], in1=bn[:], op=OP.add)
                nc.scalar.activation(hsl, hsl, AF.Relu)

            # ---------------- prologue: seg_env | seg_act ----------------
            if not DBG_PRO:
                nc.any.memset(seg_sb[:], 0.0)
            for t0, ns in (_sub_batches(NT, SUB) if DBG_PRO else []):
                eab = gat.tile([P, SUB * CPT, 8], f32, tag="eab")
                k0 = t0 * CPT
                nc.sync.dma_start(eab[:, :ns * CPT, :], ea_dram[:, k0:k0 + ns * CPT, :])
                for tl in range(ns):
                    t = t0 + tl
                    seg_ps = ps_seg.tile([P, 80], f32, tag="seg")
                    for j in range(CPT):
                        k = t * CPT + j
                        EA = ea_chunk(eab, tl * CPT + j, k)
                        oh = onehot(k)
                        nc.tensor.matmul(seg_ps[:], oh[:], EA[:],
                                         start=(j == 0), stop=(j == CPT - 1))
                    nc.vector.tensor_copy(seg_sb[:, t * 80:(t + 1) * 80], seg_ps[:])

            # ---------------- layers ----------------
            for li in range(DBG_LAYERS):
                for t in range(NT):
                    ln_tile(t, hn_sb)
                contrib_dma(ag1_in, hn_sb, 64, 64)
                nc.gpsimd.collective_compute(
                    "AllGather", mybir.AluOpType.bypass,
                    ins=[ag1_in[:]], outs=[hn_tab[:]], replica_groups=groups)

                # pass 1: action-net conv1 (shared agg) -> a_in|a_out
                if "1" not in DBG_PASSES:
                    nc.any.memset(aio_sb[:], 0.1)
                for t0, ns in (_sub_batches(NT, SUB) if "1" in DBG_PASSES else []):
                    dstA = gat.tile([P, SUB * CA, 64], f32, tag="gA")
                    nA = ns * CA * 128
                    offA = t0 * CA * 128 // 16
                    nc.gpsimd.dma_gather(
                        dstA[:, :ns * CA, :], hn_tab[:],
                        idxA[:, offA: offA + nA // 16], nA, nreg[nA], 64, single_packet=False)
                    dstB = gat.tile([P, SUB * CB, 64], f32, tag="gB")
                    nB = ns * CB * 128
                    offB = t0 * CB * 128 // 16
                    nc.gpsimd.dma_gather(
                        dstB[:, :ns * CB, :], hn_tab[SPLIT:, :],
                        idxB[:, offB: offB + nB // 16], nB, nreg[nB], 64, single_packet=False)
                    for tl in range(ns):
                        t = t0 + tl
                        agg_ps = ps_seg.tile([P, 80], f32, tag="seg")
                        for j in range(CA):
                            oh = onehot(t * CPT + j)
                            nc.tensor.matmul(agg_ps[:, :64], oh[:],
                                             dstA[:, tl * CA + j, :],
                                             start=(j == 0), stop=False)
                        for j in range(CB):
                            oh = onehot(t * CPT + CA + j)
                            nc.tensor.matmul(agg_ps[:, :64], oh[:],
                                             dstB[:, tl * CB + j, :],
                                             start=False, stop=(j == CB - 1))
                        agg1 = wk.tile([P, 64], f32, tag="agg1")
                        nc.vector.tensor_tensor(
                            out=agg1[:], in0=agg_ps[:, :64],
                            in1=seg_sb[:, t * 80: t * 80 + 64], op=OP.add)
                        a_ps = cat_mm(hn_sb[:, t * 64:(t + 1) * 64], agg1[:], Wa[:], 32)
                        asl = aio_sb[:, t * 32:(t + 1) * 32]
                        nc.vector.tensor_tensor(out=asl, in0=a_ps[:, :32], in1=ba[:], op=OP.add)
                        nc.scalar.activation(asl, asl, AF.Relu)
                contrib_dma(ag2_in, aio_sb, 64, 32)
                nc.gpsimd.collective_compute(
                    "AllGather", mybir.AluOpType.bypass,
                    ins=[ag2_in[:]], outs=[aio_tab[:]], replica_groups=groups)

                # pass 2: action-net conv2 -> logits
                if "2" not in DBG_PASSES:
                    nc.any.memset(l4_sb[:], 0.1)
                for t0, ns in (_sub_batches(NT, SUB) if "2" in DBG_PASSES else []):
                    dstA = gat.tile([P, SUB * CA, 64], f32, tag="gA")
                    nA = ns * CA * 128
                    offA = t0 * CA * 128 // 16
                    nc.gpsimd.dma_gather(
                        dstA[:, :ns * CA, :], aio_tab[:],
                        idxA[:, offA: offA + nA // 16], nA, nreg[nA], 64, single_packet=False)
                    dstB = gat.tile([P, SUB * CB, 64], f32, tag="gB")
                    nB = ns * CB * 128
                    offB = t0 * CB * 128 // 16
                    nc.gpsimd.dma_gather(
                        dstB[:, :ns * CB, :], aio_tab[SPLIT:, :],
                        idxB[:, offB: offB + nB // 16], nB, nreg[nB], 64, single_packet=False)
                    for tl in range(ns):
                        t = t0 + tl
                        agg_ps = ps_seg.tile([P, 80], f32, tag="seg")
                        for j in range(CA):
                            oh = onehot(t * CPT + j)
                            nc.tensor.matmul(agg_ps[:, :32], oh[:],
                                             dstA[:, tl * CA + j, 0:32],
                                             start=(j == 0), stop=False)
                        for j in range(CB):
                            oh = onehot(t * CPT + CA + j)
                            nc.tensor.matmul(agg_ps[:, :32], oh[:],
                                             dstB[:, tl * CB + j, 0:32],
                                             start=False, stop=(j == CB - 1))
                        cat4 = wk.tile([P, 64], f32, tag="cat4")
                        sa = seg_sb[:, t * 80 + 64: t * 80 + 80]
                        nc.vector.tensor_copy(cat4[:, 0:16], aio_sb[:, t * 32: t * 32 + 16])
                        nc.vector.tensor_tensor(out=cat4[:, 16:32], in0=agg_ps[:, 0:16],
                                                in1=sa, op=OP.add)
                        nc.vector.tensor_copy(cat4[:, 32:48], aio_sb[:, t * 32 + 16: t * 32 + 32])
                        nc.vector.tensor_tensor(out=cat4[:, 48:64], in0=agg_ps[:, 16:32],
                                                in1=sa, op=OP.add)
                        l_ps = mm65(cat4[:], W5[:], 4)
                        nc.vector.tensor_copy(l4_sb[:, t * 4:(t + 1) * 4], l_ps[:, :4])

                # keeps
                dgin = dg6[:, (2 * li) * NT:(2 * li + 1) * NT]
                dgout = dg6[:, (2 * li + 1) * NT:(2 * li + 2) * NT]
                din_ = wk.tile([P, NT], f32, tag="din")
                nc.vector.tensor_tensor(out=din_[:], in0=l4_sb[:, 0::4],
                                        in1=l4_sb[:, 1::4], op=OP.subtract)
                nc.vector.tensor_tensor(out=din_[:], in0=din_[:], in1=dgin, op=OP.add)
                nc.vector.tensor_scalar(out=ik_sb[:], in0=din_[:], scalar1=0.0,
                                        scalar2=None, op0=OP.is_ge)
                dout_ = wk.tile([P, NT], f32, tag="dout")
                nc.vector.tensor_tensor(out=dout_[:], in0=l4_sb[:, 2::4],
                                        in1=l4_sb[:, 3::4], op=OP.subtract)
                nc.vector.tensor_tensor(out=dout_[:], in0=dout_[:], in1=dgout, op=OP.add)
                nc.vector.tensor_scalar(out=ok_sb[:], in0=dout_[:], scalar1=0.0,
                                        scalar2=None, op0=OP.is_ge)

                # wtab contribution [hn | ok | junk]
                for t in range(NT):
                    nc.vector.tensor_copy(wt_sb[:, t * 128: t * 128 + 64],
                                          hn_sb[:, t * 64:(t + 1) * 64])
                    nc.vector.tensor_copy(wt_sb[:, t * 128 + 64: t * 128 + 65],
                                          ok_sb[:, t:t + 1])
                contrib_dma(ag3_in, wt_sb, 128, 128)
                nc.gpsimd.collective_compute(
                    "AllGather", mybir.AluOpType.bypass,
                    ins=[ag3_in[:]], outs=[w_tab[:]], replica_groups=groups)

                # pass 3: weighted conv, h += relu([hn|agg_w] @ Wc + bc)
                for t0, ns in (_sub_batches(NT, SUB3) if "3" in DBG_PASSES else []):
                    dstA = gat.tile([P, SUB3 * CA, 128], f32, tag="gA")
                    nA = ns * CA * 128
                    offA = t0 * CA * 128 // 16
                    nc.gpsimd.dma_gather(
                        dstA[:, :ns * CA, :], w_tab[:],
                        idxA3[:, offA: offA + nA // 16], nA, nreg[nA], 128, single_packet=False)
                    dstB = gat.tile([P, SUB3 * CB, 128], f32, tag="gB")
                    nB = ns * CB * 128
                    offB = t0 * CB * 128 // 16
                    nc.gpsimd.dma_gather(
                        dstB[:, :ns * CB, :], w_tab[SPLIT:, :],
                        idxB3[:, offB: offB + nB // 16], nB, nreg[nB], 128, single_packet=False)
                    eab = gat.tile([P, SUB * CPT, 8], f32, tag="eab")
                    k0 = t0 * CPT
                    nc.sync.dma_start(eab[:, :ns * CPT, :], ea_dram[:, k0:k0 + ns * CPT, :])
                    for tl in range(ns):
                        t = t0 + tl
                        agg_ps = ps_seg.tile([P, 80], f32, tag="seg")
                        for j in range(CPT):
                            k = t * CPT + j
                            if j < CA:
                                gsl = dstA[:, tl * CA + j, :]
                            else:
                                gsl = dstB[:, tl * CB + (j - CA), :]
                            EA = ea_chunk(eab, tl * CPT + j, k)
                            msg = wk.tile([P, 64], f32, tag="msg")
                            nc.vector.tensor_tensor(out=msg[:], in0=gsl[0:P, 0:64],
                                                    in1=EA[:, 0:64], op=OP.add)
                            ohw = wk.tile([P, P], f32, tag="oh")
                            nc.vector.tensor_scalar(
                                out=ohw[:], in0=iota[:],
                                scalar1=lv[:, k:k + 1], scalar2=gsl[0:P, 64:65],
                                op0=OP.is_equal, op1=OP.mult)
                            nc.tensor.matmul(agg_ps[:, :64], ohw[:], msg[:],
                                             start=(j == 0), stop=(j == CPT - 1))
                        aggw = wk.tile([P, 64], f32, tag="aggw")
                        nc.vector.tensor_scalar_mul(aggw[:], agg_ps[:, :64],
                                                    ik_sb[:, t:t + 1])
                        o_ps = cat_mm(hn_sb[:, t * 64:(t + 1) * 64], aggw[:],
                                      Wc[li][:], 64)
                        outt = wk.tile([P, 64], f32, tag="outt")
                        nc.vector.tensor_tensor(out=outt[:], in0=o_ps[:, :64],
                                                in1=bc[li][:], op=OP.add)
                        nc.scalar.activation(outt[:], outt[:], AF.Relu)
                        hsl = h_sb[:, t * 64:(t + 1) * 64]
                        nc.vector.tensor_tensor(out=hsl, in0=hsl, in1=outt[:], op=OP.add)

            if DBG == "seg":
                nc.sync.dma_start(o_dbg[:], seg_sb[:])
            elif DBG == "hn":
                nc.sync.dma_start(o_dbg[:, :NT * 64], hn_sb[:])
            elif DBG == "aio":
                nc.sync.dma_start(o_dbg[:, :NT * 32], aio_sb[:])
            elif DBG == "l4":
                nc.sync.dma_start(o_dbg[:, :NT * 4], l4_sb[:])
            elif DBG == "h":
                nc.sync.dma_start(o_dbg[:, :NT * 64], h_sb[:])
            elif DBG == "keep":
                nc.sync.dma_start(o_dbg[:, :NT], ik_sb[:])
                nc.sync.dma_start(o_dbg[:, NT:2 * NT], ok_sb[:])
            else:
                nc.sync.dma_start(o_dbg[:, 0:1], ik_sb[:, 0:1])

            # ---------------- epilogue ----------------
            pool_ps = ps_pool.tile([64, 128], f32, space="PSUM")
            for t in range(NT):
                ln_tile(t, hn_sb)
                t1_ps = mm65(hn_sb[:, t * 64:(t + 1) * 64], Wd1[:], 64)
                t1 = wk.tile([P, 64], f32, tag="t1")
                nc.scalar.activation(t1[:], t1_ps[:, :64], AF.Relu)
                e_ps = mm65(t1[:], Wd2[:], 64)
                emb = wk.tile([P, 64], f32, tag="emb")
                nc.vector.tensor_copy(emb[:], e_ps[:, :64])
                t2_ps = mm65(emb[:], Wg1[:], 64)
                t2 = wk.tile([P, 64], f32, tag="t2")
                nc.scalar.activation(t2[:], t2_ps[:, :64], AF.Relu)
                g_ps = mm65(t2[:], Wg2[:], 64)
                ew = wk.tile([P, 128], f32, tag="ew")
                nc.scalar.activation(ew[:, 0:64], g_ps[:, :64], AF.Exp)
                nc.vector.tensor_tensor(out=ew[:, 64:128], in0=ew[:, 0:64],
                                        in1=emb[:], op=OP.mult)
                ohg = wk.tile([P, 64], f32, tag="ohg")
                nc.vector.tensor_tensor(out=ohg[:], in0=bt[:, t:t + 1].to_broadcast([P, 64]),
                                        in1=iota[:, 0:64], op=OP.is_equal)
                nc.tensor.matmul(pool_ps[:], ohg[:], ew[:],
                                 start=(t == 0), stop=(t == NT - 1))
                if t < 48:
                    nc.sync.dma_start(
                        o_emb[0:48 * P, :].rearrange("(tt p) f -> p tt f", p=P)[:, t:t + 1, :],
                        emb[:].rearrange("p (o f) -> p o f", o=1))
                else:
                    nc.sync.dma_start(o_emb[48 * P: 48 * P + LAST_ROWS, :],
                                      emb[0:LAST_ROWS, :])
            pool_sb = wk.tile([64, 128], f32, tag="poolsb")
            nc.vector.tensor_copy(pool_sb[:], pool_ps[:])
            nc.sync.dma_start(o_pool[:], pool_sb[:])

    lower_extended_insts(nc)
    fix_waits(nc)
    return nc


def kernel(**inp):
    import sys, os
    sys.path.insert(0, os.path.dirname(os.path.abspath(__file__)))
    if "/opt/trn_rl_repo" not in sys.path:
        sys.path.insert(0, "/opt/trn_rl_repo")
    from concourse.bass_utils import run_bass_kernel_spmd

    edge_index = np.asarray(inp["edge_index"])
    batch = np.asarray(inp["batch"])
    x = np.asarray(inp["x"], np.float32)
    ea = np.asarray(inp["edge_attr"], np.float32)

    prep = _prep(edge_index, batch)
    CA, CB, CPT, NCH = prep["CA"], prep["CB"], prep["CPT"], prep["NCH"]
    dg = _gumbel_diffs()

    f = lambda k: np.asarray(inp[k], np.float32)
    tile128 = lambda vec: np.tile(np.asarray(vec, np.float32)[None, :], (P, 1))
    W5 = np.zeros((65, 4), np.float32)
    W5[0:32, 0:2] = f("Wa2"); W5[32:64, 2:4] = f("Wo2")
    W5[64, 0:2] = f("ba2"); W5[64, 2:4] = f("bo2")
    Wn = f("Wn")
    W8 = np.zeros((8, 80), np.float32)
    W8[0:7, 0:64] = f("We_env"); W8[0:7, 64:80] = f("We_act")
    W8[7, 0:64] = f("be_env"); W8[7, 64:80] = f("be_act")
    mk65 = lambda W, b: np.concatenate([f(W), f(b)[None, :]], axis=0).astype(np.float32)

    common = {
        "iota": np.tile(np.arange(P, dtype=np.float32), (P, 1)),
        "Wn1": Wn[0:128].copy(), "Wn2": Wn[128:153].copy(),
        "bn_t": tile128(f("bn")),
        "W8": W8,
        "Wa1o1": np.concatenate([f("Wa1"), f("Wo1")], axis=1),
        "ba1o1_t": tile128(np.concatenate([f("ba1"), f("bo1")])),
        "W5": W5,
        "Wd1b": mk65("Wd1", "bd1"), "Wd2b": mk65("Wd2", "bd2"),
        "Wg1b": mk65("Wg1", "bg1"), "Wg2b": mk65("Wg2", "bg2"),
        "lng_t": tile128(f("ln_g")), "lnb_t": tile128(f("ln_b")),
    }
    for i in range(3):
        common[f"Wc{i}"] = f(f"Wc{i}")
        common[f"bc{i}_t"] = tile128(f(f"bc{i}"))

    in_maps = []
    for c in range(NC):
        pc = prep["cores"][c]
        nodes = slice(c * NPC, (c + 1) * NPC)
        xT = np.zeros((F_NODE, NT * P), np.float32)
        xT[:, :NPC] = x[nodes].T
        ea8 = np.zeros((NCH * 128, 8), np.float32)
        ea8[:, 7] = 1.0
        valid = pc["eid"] >= 0
        ea8[valid, 0:7] = ea[pc["eid"][valid]]
        dg6 = np.zeros((P, 6 * NT), np.float32)
        for li in range(3):
            for s, arr in ((0, dg[2 * li]), (1, dg[2 * li + 1])):
                block = np.zeros(NT * P, np.float32)
                block[:NPC] = arr[nodes]
                dg6[:, (2 * li + s) * NT:(2 * li + s + 1) * NT] = \
                    block.reshape(NT, P).T
        m = dict(common)
        m.update({
            "lv": pc["lv"], "idxA": pc["idxA"], "idxB": pc["idxB"],
            "idxA3": pc["idxA3"], "idxB3": pc["idxB3"],
            "ea8": ea8, "bt": pc["bt"], "dg6": dg6,
            "xT1": xT[0:128].copy(), "xT2": xT[128:153].copy(),
        })
        in_maps.append(m)

    shapes = {k: prep["cores"][0][k].shape for k in ("idxA", "idxB", "idxA3", "idxB3")}
    pkey = (CA, CB)
    if pkey not in _cache:
        _cache[pkey] = _build_program(CA, CB, shapes)
    nc = _cache[pkey]

    res = run_bass_kernel_spmd(nc, in_maps, core_ids=list(range(NC)))
    embs = []
    S = np.zeros((64, 64), np.float64)
    Pp = np.zeros((64, 64), np.float64)
    for c in range(NC):
        r = res.results[c]
        embs.append(r["o_emb"])
        S += r["o_pool"][:, 0:64]
        Pp += r["o_pool"][:, 64:128]
    graph_emb = np.concatenate(embs, axis=0).astype(np.float32)
    pooled = (Pp / S).astype(np.float32)
    return graph_emb, pooled
